# revision 28
# baseline (speedup 1.0000x reference)
"""Trainium2 Bass kernel for a pre-LN causal decoder layer (MHA + SwiGLU).

Sharding: 2-way data parallel over batch x 4-way tensor parallel over heads.
Core c (of 8): batch b=c//4, group rank r=c%4, heads [4r, 4r+4).
Each core computes Q/K/V + causal attention for its 4 heads over its batch's
2048 tokens, a partial ctx @ Wo[rows], then ReduceScatter(add) chunks (one per
512-token J-block) over the 4-core group hand each core complete attention
output for 4x128 owned tokens. The FFN (SwiGLU, full weights, fp8 resident in
SBUF with DoubleRow matmuls) runs token-parallel on those 512 tokens in two
groups so the first group overlaps the RS tail; the host concatenates shards.

LayerNorm is folded: matmuls run on raw x^T (host-transposed, bf16) and the
per-token (mean, rstd) fixup is applied to the QKV PSUM with per-partition
scalars; gamma is folded into the weights on the host.
"""

import math
import sys

sys.path.insert(0, "/opt/trn_rl_repo")

import numpy as np
import ml_dtypes

import concourse.bass as bass
import concourse.mybir as mybir
import concourse.tile as tile
from concourse import bacc
from concourse.bass_utils import run_bass_kernel_spmd
from concourse.masks import make_identity

BF16 = ml_dtypes.bfloat16
E4 = ml_dtypes.float8_e4m3fn
F32 = mybir.dt.float32
BF = mybir.dt.bfloat16
F8 = mybir.dt.float8e4

B, T, C = 2, 2048, 1024
H, HS = 16, 64
HID = 2730
HIDP = 2816  # padded to 22*128
NF = HIDP // 128  # 22
HPC = 4  # heads per core
TLOC = T // 4  # 512 tokens owned post-RS
EPS = 1e-3
NEG = -60.0
RG = [[0, 1, 2, 3], [4, 5, 6, 7]]
NT = T // 128  # 16 token tiles
NJ = T // 512  # 4 t-blocks of 512
NKC = C // 128  # 8 contraction chunks
CHUNK_T = [(512 * j, 512 * j + 512) for j in range(4)]
CHUNK_O = [(128 * j, 128 * j + 128) for j in range(4)]
S_X = 16.0  # hn2 fp8 quant scale
S_G = 16.0  # g fp8 quant scale

_cache = {}


def _build(have_bw, s_w, s_w3, sim=False):
    nc = bacc.Bacc(None, target_bir_lowering=False, debug=False)
    xT = nc.declare_dram_parameter("xT", [C, T], BF, isOutput=False)
    xres = nc.declare_dram_parameter("xres", [TLOC, C], F32, isOutput=False)
    wqkv = nc.declare_dram_parameter("wqkv", [C, 768], BF, isOutput=False)
    gws = nc.declare_dram_parameter("gws", [768], F32, isOutput=False)
    if have_bw:
        bw = nc.declare_dram_parameter("bw", [768], F32, isOutput=False)
        bw1 = nc.declare_dram_parameter("bw1", [HIDP], F32, isOutput=False)
        bw2 = nc.declare_dram_parameter("bw2", [HIDP], F32, isOutput=False)
    wo = nc.declare_dram_parameter("wo", [256, C], BF, isOutput=False)
    wf12 = nc.declare_dram_parameter("wf12", [NF, 128, 2048], F8, isOutput=False)
    wf3 = nc.declare_dram_parameter("wf3", [NF, 128, 1024], F8, isOutput=False)
    out = nc.declare_dram_parameter("out", [TLOC, C], F32, isOutput=True)

    rs_in = nc.dram_tensor("rs_in", [T, C], BF)
    rs_out = nc.dram_tensor("rs_out", [TLOC, C], BF)

    with tile.TileContext(nc) as tc:
        from contextlib import ExitStack
        with ExitStack() as ctx:
            consts = ctx.enter_context(tc.tile_pool(name="consts", bufs=1))
            ident = consts.tile([128, 128], BF)
            make_identity(nc, ident)
            maskc = consts.tile([128, 128], F32)
            nc.gpsimd.memset(maskc, 0.0)
            # keep where col >= row (s <= t), else NEG
            nc.gpsimd.affine_select(
                out=maskc, in_=maskc, compare_op=mybir.AluOpType.is_ge,
                fill=NEG, base=0, pattern=[[1, 128]], channel_multiplier=-1)
            epsc = consts.tile([128, 1], F32)
            nc.vector.memset(epsc, EPS)
            eps2c = consts.tile([128, 1], F32)
            nc.vector.memset(eps2c, EPS / (S_X * S_X))
            gws_row = consts.tile([1, 768], F32)
            nc.sync.dma_start(out=gws_row, in_=gws[:].rearrange("(o j) -> o j", o=1))
            if have_bw:
                bw_cols = consts.tile([128, 6], F32)
                nc.sync.dma_start(out=bw_cols, in_=bw[:].rearrange("(j p) -> p j", p=128))
                bw1c = consts.tile([128, NF], F32)
                nc.sync.dma_start(out=bw1c, in_=bw1[:].rearrange("(f p) -> p f", p=128))
                bw2c = consts.tile([128, NF], F32)
                nc.sync.dma_start(out=bw2c, in_=bw2[:].rearrange("(f p) -> p f", p=128))
            wqkv_sb = consts.tile([128, NKC, 768], BF)
            nc.sync.dma_start(out=wqkv_sb, in_=wqkv[:].rearrange("(k p) j -> p k j", p=128))
            wo_sb = consts.tile([128, 2, C], BF)
            nc.sync.dma_start(out=wo_sb, in_=wo[:].rearrange("(k p) j -> p k j", p=128))

            # ---------------- Phase A: row-stats + fused-LN QKV (qkv^T) -----
            pA = ctx.enter_context(tc.tile_pool(name="pA", bufs=1))
            qT2 = pA.tile([128, 2, T], BF)   # [256 q-dims, T]
            kT2 = pA.tile([128, 2, T], BF)
            v_sb = pA.tile([128, NT, HPC, 65], BF)
            nc.vector.memset(v_sb[:, :, :, 64:65], 1.0)

            with tc.tile_pool(name="xTp", bufs=1) as xTp, \
                 tc.tile_pool(name="vTp", bufs=1) as vTp, \
                 tc.tile_pool(name="rowp", bufs=1) as rowp, \
                 tc.tile_pool(name="xsqp", bufs=1) as xsqp, \
                 tc.tile_pool(name="strow", bufs=4) as strow, \
                 tc.tile_pool(name="psS", bufs=1, space="PSUM") as psS, \
                 tc.tile_pool(name="psA", bufs=3, space="PSUM") as psA, \
                 tc.tile_pool(name="psV", bufs=3, space="PSUM") as psV:
                xT_sb = xTp.tile([128, NKC, T], BF)
                xTr = xT[:].rearrange("(k p) t -> p k t", p=128)
                qs = [nc.sync, nc.scalar, nc.gpsimd]
                for tq in range(4):
                    sl = slice(tq * 512, (tq + 1) * 512)
                    for kc in range(NKC):
                        qs[(tq * NKC + kc) % 3].dma_start(
                            out=xT_sb[:, kc, sl], in_=xTr[:, kc, sl])
                vT_tmp = vTp.tile([128, 2, T], BF)
                onesc = rowp.tile([128, 1], BF)
                nc.vector.memset(onesc, 1.0)
                negmu_row = rowp.tile([1, T], F32)
                rstd_row = rowp.tile([1, T], F32)
                rstd_b = rowp.tile([128, T], F32)

                # tq-outer: stats then fused-LN QKV per token quarter so the
                # pipeline ramps as soon as the first xT quarter lands
                xsq_sb = xsqp.tile([128, NKC, T], BF)
                dests = [(qT2, 0), (qT2, 1), (kT2, 0), (kT2, 1), (vT_tmp, 0), (vT_tmp, 1)]
                for tq in range(4):
                    sl = slice(tq * 512, (tq + 1) * 512)
                    for kc in range(NKC):
                        nc.vector.tensor_mul(xsq_sb[:, kc, sl], xT_sb[:, kc, sl],
                                             xT_sb[:, kc, sl])
                    mu_ps = psS.tile([1, 512], F32, tag="mu")
                    sq_ps = psS.tile([1, 512], F32, tag="sq")
                    for kc in range(NKC):
                        nc.tensor.matmul(mu_ps, onesc, xT_sb[:, kc, sl],
                                         start=(kc == 0), stop=(kc == NKC - 1))
                        nc.tensor.matmul(sq_ps, onesc, xsq_sb[:, kc, sl],
                                         start=(kc == 0), stop=(kc == NKC - 1))
                    nc.vector.tensor_scalar_mul(negmu_row[:, sl], mu_ps, -1.0 / C)
                    mu2 = strow.tile([1, 512], F32, tag="mu2")
                    nc.vector.tensor_mul(mu2, negmu_row[:, sl], negmu_row[:, sl])
                    var = strow.tile([1, 512], F32, tag="var")
                    nc.vector.tensor_scalar_mul(var, sq_ps, 1.0 / C)
                    nc.vector.tensor_sub(var, var, mu2)
                    sd = strow.tile([1, 512], F32, tag="sd")
                    nc.scalar.activation(out=sd, in_=var,
                                         func=mybir.ActivationFunctionType.Sqrt,
                                         bias=epsc[0:1, 0:1])
                    nc.vector.reciprocal_approx_fast(out=rstd_row[:, sl], in_=sd)
                    nc.gpsimd.partition_broadcast(rstd_b[:, sl], rstd_row[:, sl])
                    for jb in range(6):
                        dst, dslot = dests[jb]
                        jsl = slice(jb * 128, (jb + 1) * 128)
                        ps = psA.tile([128, 512], F32, tag="qkv")
                        for kc in range(NKC):
                            nc.tensor.matmul(ps, wqkv_sb[:, kc, jsl],
                                             xT_sb[:, kc, sl],
                                             start=(kc == 0), stop=False)
                        nc.tensor.matmul(ps, gws_row[0:1, jsl], negmu_row[0:1, sl],
                                         start=False, stop=True)
                        if have_bw:
                            tmpf = strow.tile([128, 512], F32, tag="tmpf")
                            nc.vector.tensor_mul(tmpf, ps, rstd_b[:, sl])
                            nc.vector.tensor_scalar_add(dst[:, dslot, sl], tmpf,
                                                        bw_cols[:, jb:jb + 1])
                        else:
                            nc.vector.tensor_mul(dst[:, dslot, sl], ps, rstd_b[:, sl])

                # transpose v back to [s, d] per head
                for h in range(HPC):
                    po = (h % 2) * 64
                    idn = ident[po:po + 64, po:po + 64]
                    for si in range(NT):
                        tp = psV.tile([128, 64], BF, tag="tpv")
                        nc.tensor.transpose(
                            tp, vT_tmp[po:po + 64, h // 2, si * 128:(si + 1) * 128], idn)
                        nc.vector.tensor_copy(v_sb[:, si, h, 0:64], tp)

            # ---- resident FFN weights (fp8), streamed during attention ----
            pC = ctx.enter_context(tc.tile_pool(name="pC", bufs=1))
            ctxT = pC.tile([128, 2, T], BF)  # [256 head-dims, T]
            pE = ctx.enter_context(tc.tile_pool(name="pE", bufs=1))
            out1 = pE.tile([128, 4, C], F32)
            hn2T = pE.tile([128, NKC, TLOC], F8)
            g_sb = pE.tile([128, NF, TLOC], F8)
            w12r = pE.tile([128, NF, 2048], F8)  # w1 kc-chunks | w2 kc-chunks
            w3r = pE.tile([128, NF, 1024], F8)
            xr_sb = pE.tile([128, 4, C], F32)
            # resident FFN weights: few big DMAs spread across queues, issued
            # before attention so the transfers drain during it
            wf12r = wf12[:].rearrange("f p c -> p f c")
            wf3r = wf3[:].rearrange("f p c -> p f c")
            nc.sync.dma_start(out=w12r[:, 0:11, :], in_=wf12r[:, 0:11, :])
            nc.scalar.dma_start(out=w12r[:, 11:NF, :], in_=wf12r[:, 11:NF, :])
            nc.gpsimd.dma_start(out=w3r[:, 0:11, :], in_=wf3r[:, 0:11, :])
            nc.gpsimd.dma_start(out=w3r[:, 11:NF, :], in_=wf3r[:, 11:NF, :])

            def dr_pair(ap2d, m=128):
                # [128, 2*m] AP -> [128, 2, m] view (two k-tiles for DoubleRow)
                return bass.AP(tensor=ap2d.tensor, offset=ap2d.offset,
                               ap=[list(ap2d.ap[0]), [m, 2], [1, m]])
            nc.sync.dma_start(out=xr_sb,
                              in_=xres[:].rearrange("(a p) c -> p a c", p=128))

            # ---- attention (J-outer) + Wo + per-J-block ReduceScatter ----
            with tc.tile_pool(name="scps", bufs=2, space="PSUM") as scps, \
                 tc.tile_pool(name="avps", bufs=2, space="PSUM") as avps, \
                 tc.tile_pool(name="wops", bufs=2, space="PSUM") as wops, \
                 tc.tile_pool(name="attnp", bufs=8) as attnp, \
                 tc.tile_pool(name="band", bufs=4) as bandp, \
                 tc.tile_pool(name="stC", bufs=2) as stC:

                for J in range(NJ):
                    nst = 4 * J + 4  # s-tiles 0..nst-1
                    for hp in range(2):
                        # head pair (2hp, 2hp+1) -> PE row halves 0 / 64:
                        # their score matmuls pack into disjoint PE row
                        # groups and run concurrently
                        avs = [avps.tile([65, 512], F32, tag="av", name=f"av{J}_{hp}_{hx}")
                               for hx in range(2)]

                        def emit_sc(i, po):
                            w = i - 4 * J
                            off = max(w, 0) * 128
                            sc = scps.tile([128, 512], F32, tag="sc")
                            nc.tensor.matmul(
                                sc[:, off:512],
                                kT2[po:po + 64, hp, i * 128:(i + 1) * 128],
                                qT2[po:po + 64, hp, J * 512 + off:(J + 1) * 512],
                                start=True, stop=True)
                            if w >= 0:
                                nc.vector.tensor_add(sc[:, off:off + 128],
                                                     sc[:, off:off + 128], maskc)
                            bd = bandp.tile([128, 512], BF, tag="bd")
                            nc.scalar.activation(out=bd[:, off:512], in_=sc[:, off:512],
                                                 func=mybir.ActivationFunctionType.Exp)
                            return bd, off

                        def emit_av(i, hx, bd_off):
                            bd, off = bd_off
                            nc.tensor.matmul(
                                avs[hx][:, off:512],
                                v_sb[:, i, 2 * hp + hx, :],
                                bd[:, off:512],
                                start=(i == 0), stop=(i == nst - 1))

                        pend = [emit_sc(0, 0), emit_sc(0, 64)]
                        for i in range(1, nst):
                            nxt0 = emit_sc(i, 0)
                            nxt1 = emit_sc(i, 64)
                            emit_av(i - 1, 0, pend[0])
                            emit_av(i - 1, 1, pend[1])
                            pend = [nxt0, nxt1]
                        emit_av(nst - 1, 0, pend[0])
                        emit_av(nst - 1, 1, pend[1])
                        # normalize: rows 0..63 are ctx^T, row 64 is denom
                        # (custom-DVE ops can't read PSUM at base_partition>0:
                        # stage the denom row to SBUF, reciprocal in place)
                        for hx in range(2):
                            po = hx * 64
                            rrow = stC.tile([1, 512], F32, tag="rr")
                            nc.vector.tensor_copy(rrow, avs[hx][64:65, :])
                            nc.vector.reciprocal_approx_fast(out=rrow, in_=rrow)
                            rb = stC.tile([64, 512], F32, tag="rb")
                            nc.gpsimd.partition_broadcast(rb[:, :], rrow[:, :])
                            nc.vector.tensor_mul(
                                ctxT[po:po + 64, hp, J * 512:(J + 1) * 512],
                                avs[hx][0:64, :], rb)
                    # Wo for this J-block's 4 token tiles, then RS chunk J
                    for ti in range(J * 4, J * 4 + 4):
                        wp = wops.tile([128, C], F32, tag="wp")
                        for dc in range(2):
                            for n0, n1 in ((0, 512), (512, 1024)):
                                nc.tensor.matmul(wp[:, n0:n1],
                                                 ctxT[:, dc, ti * 128:(ti + 1) * 128],
                                                 wo_sb[:, dc, n0:n1],
                                                 start=(dc == 0), stop=(dc == 1))
                        at = attnp.tile([128, C], BF, tag="at")
                        nc.vector.tensor_copy(at, wp)
                        nc.sync.dma_start(
                            out=rs_in[ti * 128:(ti + 1) * 128, :], in_=at)
                    t0, t1 = CHUNK_T[J]
                    o0, o1 = CHUNK_O[J]
                    if sim:
                        nc.sync.dma_start(
                            out=rs_out[o0:o1, :],
                            in_=rs_in[t0:t0 + (o1 - o0), :])
                    else:
                        nc.gpsimd.collective_compute(
                            "ReduceScatter", mybir.AluOpType.add,
                            replica_groups=RG,
                            ins=[rs_in[t0:t1, :]],
                            outs=[rs_out[o0:o1, :]])

            # ------- Phase E: LN2 prep + SwiGLU FFN in two token groups -----
            with tc.tile_pool(name="stE", bufs=2) as stE, \
                 tc.tile_pool(name="rsp", bufs=2) as rsp, \
                 tc.tile_pool(name="psE", bufs=2, space="PSUM") as psE, \
                 tc.tile_pool(name="gtmp", bufs=3) as gtmp, \
                 tc.tile_pool(name="psG", bufs=2, space="PSUM") as psG, \
                 tc.tile_pool(name="psW3", bufs=1, space="PSUM") as psW3:

                def emit_eprep(ck):
                    o0, o1 = CHUNK_O[ck]
                    tt = ck
                    rs_sb = rsp.tile([128, C], BF, tag="rs")
                    nc.gpsimd.dma_start(
                        out=rs_sb,
                        in_=rs_out[o0:o1, :].rearrange("(a p) c -> p a c", p=128))
                    o1s = out1[:, tt, :]
                    nc.vector.tensor_add(o1s, xr_sb[:, tt, :], rs_sb)
                    st = stE.tile([128, 2, 6], F32, tag="st")
                    nc.vector.bn_stats(out=st[:, 0, :], in_=o1s[:, 0:512])
                    nc.vector.bn_stats(out=st[:, 1, :], in_=o1s[:, 512:1024])
                    mv = stE.tile([128, 2], F32, tag="mv")
                    nc.vector.bn_aggr(out=mv, in_=st)
                    # sd_s = sqrt(var+eps)/S_X via activation scale/bias fold
                    sd = stE.tile([128, 1], F32, tag="sd")
                    nc.scalar.activation(out=sd, in_=mv[:, 1:2],
                                         func=mybir.ActivationFunctionType.Sqrt,
                                         bias=eps2c[:, 0:1], scale=1.0 / (S_X * S_X))
                    rstd = stE.tile([128, 1], F32, tag="rstd")
                    nc.vector.reciprocal(rstd, sd)  # = S_X * true rstd
                    rmu = stE.tile([128, 1], F32, tag="rmu")
                    nc.vector.tensor_mul(rmu, mv[:, 0:1], rstd)
                    hn2 = stE.tile([128, C], BF, tag="hn2")
                    nc.vector.tensor_scalar(hn2, o1s, rstd, rmu,
                                            mybir.AluOpType.mult,
                                            mybir.AluOpType.subtract)
                    for kc in range(NKC):
                        tp = psE.tile([128, 128], BF, tag="tpE")
                        nc.tensor.transpose(tp, hn2[:, kc * 128:(kc + 1) * 128],
                                            ident)
                        nc.vector.tensor_scalar_mul(
                            hn2T[:, kc, tt * 128:(tt + 1) * 128], tp, 1.0)

                def emit_ffn_group(g, tts, c0, c1):
                    ncols = c1 - c0
                    acc0f = psW3.tile([128, 2, 512], F32, tag="acc0", name=f"acc0_{g}")
                    acc1f = psW3.tile([128, 2, 512], F32, tag="acc1", name=f"acc1_{g}")
                    acc0 = acc0f[:, 0:len(tts), :]
                    acc1 = acc1f[:, 0:len(tts), :]
                    for fi in range(NF):
                        g12 = psG.tile([128, 2, ncols], F32, tag="g12")
                        g1 = g12[:, 0, :]
                        g2 = g12[:, 1, :]
                        for kcp in range(4):
                            nc.tensor.matmul(
                                g1, dr_pair(w12r[:, fi, kcp * 256:(kcp + 1) * 256]),
                                hn2T[:, 2 * kcp:2 * kcp + 2, c0:c1],
                                start=(kcp == 0), stop=(kcp == 3),
                                perf_mode=mybir.MatmulPerfMode.DoubleRow)
                        sil = gtmp.tile([128, ncols], BF, tag="sil")
                        nc.scalar.activation(out=sil, in_=g1,
                                             func=mybir.ActivationFunctionType.Silu,
                                             scale=1.0 / (s_w * S_X),
                                             bias=(bw1c[:, fi:fi + 1] if have_bw else 0.0))
                        for kcp in range(4):
                            nc.tensor.matmul(
                                g2, dr_pair(w12r[:, fi, 1024 + kcp * 256:
                                                 1024 + (kcp + 1) * 256]),
                                hn2T[:, 2 * kcp:2 * kcp + 2, c0:c1],
                                start=(kcp == 0), stop=(kcp == 3),
                                perf_mode=mybir.MatmulPerfMode.DoubleRow)
                        if have_bw:
                            nc.vector.tensor_scalar_add(g2, g2, bw2c[:, fi:fi + 1])
                        nc.vector.scalar_tensor_tensor(
                            out=g_sb[:, fi, c0:c1], in0=g2,
                            scalar=S_G / (s_w * S_X), in1=sil,
                            op0=mybir.AluOpType.mult, op1=mybir.AluOpType.mult)
                        if fi % 2 == 1:
                            for j, tt in enumerate(tts):
                                lhs = g_sb[:, fi - 1:fi + 1,
                                           tt * 128:(tt + 1) * 128]
                                nc.tensor.matmul(
                                    acc0[:, j, :], lhs, w3r[:, fi - 1:fi + 1, 0:512],
                                    start=(fi == 1), stop=(fi == NF - 1),
                                    perf_mode=mybir.MatmulPerfMode.DoubleRow)
                                nc.tensor.matmul(
                                    acc1[:, j, :], lhs, w3r[:, fi - 1:fi + 1, 512:1024],
                                    start=(fi == 1), stop=(fi == NF - 1),
                                    perf_mode=mybir.MatmulPerfMode.DoubleRow)
                    inv = 1.0 / (S_G * s_w3)
                    for j, tt in enumerate(tts):
                        nc.vector.scalar_tensor_tensor(
                            out=xr_sb[:, tt, 0:512], in0=acc0[:, j, :], scalar=inv,
                            in1=out1[:, tt, 0:512],
                            op0=mybir.AluOpType.mult, op1=mybir.AluOpType.add)
                        nc.vector.scalar_tensor_tensor(
                            out=xr_sb[:, tt, 512:1024], in0=acc1[:, j, :], scalar=inv,
                            in1=out1[:, tt, 512:1024],
                            op0=mybir.AluOpType.mult, op1=mybir.AluOpType.add)
                    outr = out[:].rearrange("(a p) c -> p a c", p=128)
                    nc.sync.dma_start(out=outr[:, tts[0]:tts[-1] + 1, :],
                                      in_=xr_sb[:, tts[0]:tts[-1] + 1, :])

                # group 0 fills the RS2/RS3 latency window; tt=2 follows
                # (RS2 done by then); tt=3 last (gated by RS3)
                emit_eprep(0)
                emit_eprep(1)
                emit_eprep(2)
                emit_ffn_group(0, (0, 1), 0, 256)
                emit_ffn_group(1, (2,), 256, 384)
                emit_eprep(3)
                emit_ffn_group(2, (3,), 384, 512)
    nc.compile()
    return nc


def _prep(x, Wq, Wk, Wv, Wo, W1, W2, W3, gamma, beta):
    f32 = np.float32
    scale = f32(1.0 / np.sqrt(HS))
    gcol = gamma.astype(f32)[:, None]
    w1p = np.zeros((C, HIDP), f32)
    w1p[:, :HID] = W1
    w2p = np.zeros((C, HIDP), f32)
    w2p[:, :HID] = W2
    w3p = np.zeros((HIDP, C), f32)
    w3p[:HID, :] = W3
    w1g = (gcol * w1p).reshape(NKC, 128, NF, 128).transpose(2, 1, 0, 3).reshape(NF, 128, C)
    w2g = (gcol * w2p).reshape(NKC, 128, NF, 128).transpose(2, 1, 0, 3).reshape(NF, 128, C)
    w12 = np.concatenate([w1g, w2g], axis=2)  # [NF, 128, 2048]
    s_w = float(2.0 ** math.floor(math.log2(240.0 / max(np.abs(w12).max(), 1e-30))))
    wf12 = (w12 * s_w).astype(E4)
    w3r = w3p.reshape(NF, 128, C)
    s_w3 = float(2.0 ** math.floor(math.log2(240.0 / max(np.abs(w3r).max(), 1e-30))))
    wf3 = (w3r * s_w3).astype(E4)
    bw1 = (beta.astype(f32) @ w1p).astype(f32)
    bw2 = ((beta.astype(f32) @ w2p) * (s_w * S_X)).astype(f32)
    in_maps = []
    for c in range(8):
        b, r = c // 4, c % 4
        hh = slice(r * HPC, (r + 1) * HPC)
        # per-head [C, HS] blocks -> [C, 256] column groups
        qc = Wq[hh].transpose(1, 0, 2).reshape(C, 256).astype(f32) * scale
        kc = Wk[hh].transpose(1, 0, 2).reshape(C, 256).astype(f32)
        vc = Wv[hh].transpose(1, 0, 2).reshape(C, 256).astype(f32)
        wcat = np.concatenate([qc, kc, vc], axis=1)  # [C, 768], scale folded in q
        wq_g = gcol * wcat
        gws = wq_g.sum(axis=0).astype(f32)
        bwq = (beta.astype(f32) @ wcat).astype(f32)
        xb = x[b].astype(f32)
        xres_rows = np.concatenate(
            [xb[512 * J + 128 * r: 512 * J + 128 * (r + 1)] for J in range(4)])
        m = {
            "xT": np.ascontiguousarray(xb.T).astype(BF16),
            "xres": np.ascontiguousarray(xres_rows),
            "wqkv": wq_g.astype(BF16),
            "gws": gws,
            "wo": np.ascontiguousarray(Wo[r * 256:(r + 1) * 256, :]).astype(BF16),
            "wf12": wf12,
            "wf3": wf3,
        }
        have_bw = bool(np.any(beta != 0))
        if have_bw:
            m["bw"] = bwq
            m["bw1"] = bw1
            m["bw2"] = bw2
        in_maps.append(m)
    return in_maps, have_bw, s_w, s_w3


def kernel(x, Wq, Wk, Wv, Wo, W1, W2, W3, gamma, beta, _bench=None):
    x = np.asarray(x)
    in_maps, have_bw, s_w, s_w3 = _prep(
        np.asarray(x), np.asarray(Wq), np.asarray(Wk),
        np.asarray(Wv), np.asarray(Wo), np.asarray(W1),
        np.asarray(W2), np.asarray(W3),
        np.asarray(gamma), np.asarray(beta))
    key = ("k", have_bw, s_w, s_w3)
    if key not in _cache:
        _cache[key] = _build(have_bw, s_w, s_w3)
    nc = _cache[key]
    kw = dict(_bench) if _bench else {}
    res = run_bass_kernel_spmd(nc, in_maps, list(range(8)), **kw)
    outf = np.empty((B, T, C), np.float32)
    for c in range(8):
        b, r = c // 4, c % 4
        o = res.results[c]["out"]
        for J in range(4):
            outf[b, 512 * J + 128 * r: 512 * J + 128 * (r + 1)] = \
                o[J * 128:(J + 1) * 128]
    if _bench is not None:
        kernel.last_results = res
    return outf


# revision 30
# speedup vs baseline: 1.1493x; 1.1493x over previous
"""Trainium2 Bass kernel for a pre-LN causal decoder layer (MHA + SwiGLU).

Sharding: 2-way data parallel over batch x 4-way tensor parallel over heads.
Core c (of 8): batch b=c//4, group rank r=c%4, heads [4r, 4r+4).
Each core computes Q/K/V + causal attention for its 4 heads over its batch's
2048 tokens, a partial ctx @ Wo[rows], then ReduceScatter(add) chunks (one per
512-token J-block) over the 4-core group hand each core complete attention
output for 4x128 owned tokens. The FFN (SwiGLU, full weights, fp8 resident in
SBUF with DoubleRow matmuls) runs token-parallel on those 512 tokens in two
groups so the first group overlaps the RS tail; the host concatenates shards.

LayerNorm is folded: matmuls run on raw x^T (host-transposed, bf16) and the
per-token (mean, rstd) fixup is applied to the QKV PSUM with per-partition
scalars; gamma is folded into the weights on the host.
"""

import math
import sys

sys.path.insert(0, "/opt/trn_rl_repo")

import numpy as np
import ml_dtypes

import concourse.bass as bass
import concourse.mybir as mybir
import concourse.tile as tile
from concourse import bacc
from concourse.bass_utils import run_bass_kernel_spmd
from concourse.masks import make_identity

BF16 = ml_dtypes.bfloat16
E4 = ml_dtypes.float8_e4m3fn
F32 = mybir.dt.float32
BF = mybir.dt.bfloat16
F8 = mybir.dt.float8e4

B, T, C = 2, 2048, 1024
H, HS = 16, 64
HID = 2730
HIDP = 2816  # padded to 22*128
NF = HIDP // 128  # 22
HPC = 4  # heads per core
TLOC = T // 4  # 512 tokens owned post-RS
EPS = 1e-3
NEG = -60.0
RG = [[0, 1, 2, 3], [4, 5, 6, 7]]
NT = T // 128  # 16 token tiles
NJ = T // 512  # 4 t-blocks of 512
NKC = C // 128  # 8 contraction chunks
CHUNK_T = [(512 * j, 512 * j + 512) for j in range(4)]
CHUNK_O = [(128 * j, 128 * j + 128) for j in range(4)]
S_X = 16.0  # hn2 fp8 quant scale
S_G = 16.0  # g fp8 quant scale

_cache = {}


def _build(have_bw, s_w, s_w3, sim=False):
    nc = bacc.Bacc(None, target_bir_lowering=False, debug=False)
    xT = nc.declare_dram_parameter("xT", [C, T], BF, isOutput=False)
    xres = nc.declare_dram_parameter("xres", [TLOC, C], F32, isOutput=False)
    wqkv = nc.declare_dram_parameter("wqkv", [C, 768], BF, isOutput=False)
    gws = nc.declare_dram_parameter("gws", [768], F32, isOutput=False)
    if have_bw:
        bw = nc.declare_dram_parameter("bw", [768], F32, isOutput=False)
        bw1 = nc.declare_dram_parameter("bw1", [HIDP], F32, isOutput=False)
        bw2 = nc.declare_dram_parameter("bw2", [HIDP], F32, isOutput=False)
    wo = nc.declare_dram_parameter("wo", [256, C], BF, isOutput=False)
    wf12 = nc.declare_dram_parameter("wf12", [NF, 128, 2048], F8, isOutput=False)
    wf3 = nc.declare_dram_parameter("wf3", [NF, 128, 1024], F8, isOutput=False)
    out = nc.declare_dram_parameter("out", [TLOC, C], F32, isOutput=True)

    rs_in = nc.dram_tensor("rs_in", [T, C], BF)
    rs_out = nc.dram_tensor("rs_out", [TLOC, C], BF)

    with tile.TileContext(nc) as tc:
        from contextlib import ExitStack
        with ExitStack() as ctx:
            consts = ctx.enter_context(tc.tile_pool(name="consts", bufs=1))
            ident = consts.tile([128, 128], BF)
            make_identity(nc, ident)
            maskc = consts.tile([128, 128], F32)
            nc.gpsimd.memset(maskc, 0.0)
            # keep where col >= row (s <= t), else NEG
            nc.gpsimd.affine_select(
                out=maskc, in_=maskc, compare_op=mybir.AluOpType.is_ge,
                fill=NEG, base=0, pattern=[[1, 128]], channel_multiplier=-1)
            epsc = consts.tile([128, 1], F32)
            nc.vector.memset(epsc, EPS)
            eps2c = consts.tile([128, 1], F32)
            nc.vector.memset(eps2c, EPS / (S_X * S_X))
            gws_row = consts.tile([1, 768], F32)
            nc.sync.dma_start(out=gws_row, in_=gws[:].rearrange("(o j) -> o j", o=1))
            if have_bw:
                bw_cols = consts.tile([128, 6], F32)
                nc.sync.dma_start(out=bw_cols, in_=bw[:].rearrange("(j p) -> p j", p=128))
                bw1c = consts.tile([128, NF], F32)
                nc.sync.dma_start(out=bw1c, in_=bw1[:].rearrange("(f p) -> p f", p=128))
                bw2c = consts.tile([128, NF], F32)
                nc.sync.dma_start(out=bw2c, in_=bw2[:].rearrange("(f p) -> p f", p=128))
            wqkv_sb = consts.tile([128, NKC, 768], BF)
            nc.sync.dma_start(out=wqkv_sb, in_=wqkv[:].rearrange("(k p) j -> p k j", p=128))
            wo_sb = consts.tile([128, 2, C], BF)
            nc.sync.dma_start(out=wo_sb, in_=wo[:].rearrange("(k p) j -> p k j", p=128))

            # ---------------- Phase A: row-stats + fused-LN QKV (qkv^T) -----
            pA = ctx.enter_context(tc.tile_pool(name="pA", bufs=1))
            qT2 = pA.tile([128, 2, T], BF)   # [256 q-dims, T]
            kT2 = pA.tile([128, 2, T], BF)
            v_sb = pA.tile([128, NT, HPC, 65], BF)
            nc.vector.memset(v_sb[:, :, :, 64:65], 1.0)

            with tc.tile_pool(name="xTp", bufs=1) as xTp, \
                 tc.tile_pool(name="vTp", bufs=1) as vTp, \
                 tc.tile_pool(name="rowp", bufs=1) as rowp, \
                 tc.tile_pool(name="xsqp", bufs=1) as xsqp, \
                 tc.tile_pool(name="strow", bufs=4) as strow, \
                 tc.tile_pool(name="psS", bufs=1, space="PSUM") as psS, \
                 tc.tile_pool(name="psA", bufs=3, space="PSUM") as psA, \
                 tc.tile_pool(name="psV", bufs=3, space="PSUM") as psV:
                xT_sb = xTp.tile([128, NKC, T], BF)
                xTr = xT[:].rearrange("(k p) t -> p k t", p=128)
                qs = [nc.sync, nc.scalar, nc.gpsimd]
                for tq in range(4):
                    sl = slice(tq * 512, (tq + 1) * 512)
                    for kc in range(NKC):
                        qs[(tq * NKC + kc) % 3].dma_start(
                            out=xT_sb[:, kc, sl], in_=xTr[:, kc, sl])
                vT_tmp = vTp.tile([128, 2, T], BF)
                onesc = rowp.tile([128, 1], BF)
                nc.vector.memset(onesc, 1.0)
                negmu_row = rowp.tile([1, T], F32)
                rstd_row = rowp.tile([1, T], F32)
                rstd_b = rowp.tile([128, T], F32)

                # tq-outer: stats then fused-LN QKV per token quarter so the
                # pipeline ramps as soon as the first xT quarter lands
                xsq_sb = xsqp.tile([128, NKC, T], BF)
                dests = [(qT2, 0), (qT2, 1), (kT2, 0), (kT2, 1), (vT_tmp, 0), (vT_tmp, 1)]
                for tq in range(4):
                    sl = slice(tq * 512, (tq + 1) * 512)
                    for kc in range(NKC):
                        nc.vector.tensor_mul(xsq_sb[:, kc, sl], xT_sb[:, kc, sl],
                                             xT_sb[:, kc, sl])
                    mu_ps = psS.tile([1, 512], F32, tag="mu")
                    sq_ps = psS.tile([1, 512], F32, tag="sq")
                    for kc in range(NKC):
                        nc.tensor.matmul(mu_ps, onesc, xT_sb[:, kc, sl],
                                         start=(kc == 0), stop=(kc == NKC - 1))
                        nc.tensor.matmul(sq_ps, onesc, xsq_sb[:, kc, sl],
                                         start=(kc == 0), stop=(kc == NKC - 1))
                    nc.vector.tensor_scalar_mul(negmu_row[:, sl], mu_ps, -1.0 / C)
                    mu2 = strow.tile([1, 512], F32, tag="mu2")
                    nc.vector.tensor_mul(mu2, negmu_row[:, sl], negmu_row[:, sl])
                    var = strow.tile([1, 512], F32, tag="var")
                    nc.vector.tensor_scalar_mul(var, sq_ps, 1.0 / C)
                    nc.vector.tensor_sub(var, var, mu2)
                    sd = strow.tile([1, 512], F32, tag="sd")
                    nc.scalar.activation(out=sd, in_=var,
                                         func=mybir.ActivationFunctionType.Sqrt,
                                         bias=epsc[0:1, 0:1])
                    nc.vector.reciprocal_approx_fast(out=rstd_row[:, sl], in_=sd)
                    nc.gpsimd.partition_broadcast(rstd_b[:, sl], rstd_row[:, sl])
                    for jb in range(6):
                        dst, dslot = dests[jb]
                        jsl = slice(jb * 128, (jb + 1) * 128)
                        ps = psA.tile([128, 512], F32, tag="qkv")
                        for kc in range(NKC):
                            nc.tensor.matmul(ps, wqkv_sb[:, kc, jsl],
                                             xT_sb[:, kc, sl],
                                             start=(kc == 0), stop=False)
                        nc.tensor.matmul(ps, gws_row[0:1, jsl], negmu_row[0:1, sl],
                                         start=False, stop=True)
                        if have_bw:
                            tmpf = strow.tile([128, 512], F32, tag="tmpf")
                            nc.vector.tensor_mul(tmpf, ps, rstd_b[:, sl])
                            nc.vector.tensor_scalar_add(dst[:, dslot, sl], tmpf,
                                                        bw_cols[:, jb:jb + 1])
                        else:
                            nc.vector.tensor_mul(dst[:, dslot, sl], ps, rstd_b[:, sl])

                # transpose v back to [s, d] per head
                for h in range(HPC):
                    po = (h % 2) * 64
                    idn = ident[po:po + 64, po:po + 64]
                    for si in range(NT):
                        tp = psV.tile([128, 64], BF, tag="tpv")
                        nc.tensor.transpose(
                            tp, vT_tmp[po:po + 64, h // 2, si * 128:(si + 1) * 128], idn)
                        nc.vector.tensor_copy(v_sb[:, si, h, 0:64], tp)

            # ---- resident FFN weights (fp8), streamed during attention ----
            pC = ctx.enter_context(tc.tile_pool(name="pC", bufs=1))
            ctxT = pC.tile([128, 2, T], BF)  # [256 head-dims, T]
            pE = ctx.enter_context(tc.tile_pool(name="pE", bufs=1))
            out1 = pE.tile([128, 4, C], F32)
            hn2T = pE.tile([128, NKC, TLOC], F8)
            g_sb = pE.tile([128, NF, TLOC], F8)
            w12r = pE.tile([128, NF, 2048], F8)  # w1 kc-chunks | w2 kc-chunks
            w3r = pE.tile([128, NF, 1024], F8)
            xr_sb = pE.tile([128, 4, C], F32)
            # resident FFN weights: few big DMAs spread across queues, issued
            # before attention so the transfers drain during it
            wf12r = wf12[:].rearrange("f p c -> p f c")
            wf3r = wf3[:].rearrange("f p c -> p f c")
            nc.sync.dma_start(out=w12r[:, 0:11, :], in_=wf12r[:, 0:11, :])
            nc.scalar.dma_start(out=w12r[:, 11:NF, :], in_=wf12r[:, 11:NF, :])
            nc.gpsimd.dma_start(out=w3r[:, 0:11, :], in_=wf3r[:, 0:11, :])
            nc.gpsimd.dma_start(out=w3r[:, 11:NF, :], in_=wf3r[:, 11:NF, :])

            def dr_pair(ap2d, m=128):
                # [128, 2*m] AP -> [128, 2, m] view (two k-tiles for DoubleRow)
                return bass.AP(tensor=ap2d.tensor, offset=ap2d.offset,
                               ap=[list(ap2d.ap[0]), [m, 2], [1, m]])
            nc.sync.dma_start(out=xr_sb,
                              in_=xres[:].rearrange("(a p) c -> p a c", p=128))

            # ---- attention (J-outer) + Wo + per-J-block ReduceScatter ----
            with tc.tile_pool(name="scps", bufs=2, space="PSUM") as scps, \
                 tc.tile_pool(name="avps", bufs=2, space="PSUM") as avps, \
                 tc.tile_pool(name="wops", bufs=2, space="PSUM") as wops, \
                 tc.tile_pool(name="attnp", bufs=8) as attnp, \
                 tc.tile_pool(name="band", bufs=4) as bandp, \
                 tc.tile_pool(name="stC", bufs=2) as stC:

                for J in range(NJ):
                    nst = 4 * J + 4  # s-tiles 0..nst-1
                    for hp in range(2):
                        # head pair (2hp, 2hp+1) -> PE row halves 0 / 64:
                        # their score matmuls pack into disjoint PE row
                        # groups and run concurrently
                        avs = [avps.tile([65, 512], F32, tag="av", name=f"av{J}_{hp}_{hx}")
                               for hx in range(2)]

                        def emit_sc(i, po):
                            w = i - 4 * J
                            off = max(w, 0) * 128
                            sc = scps.tile([128, 512], F32, tag="sc")
                            nc.tensor.matmul(
                                sc[:, off:512],
                                kT2[po:po + 64, hp, i * 128:(i + 1) * 128],
                                qT2[po:po + 64, hp, J * 512 + off:(J + 1) * 512],
                                start=True, stop=True)
                            bd = bandp.tile([128, 512], BF, tag="bd")
                            # exp the raw scores (keeps Vector off the exp
                            # path); zero the diagonal tile's upper triangle
                            # afterwards on GpSimd
                            nc.scalar.activation(out=bd[:, off:512], in_=sc[:, off:512],
                                                 func=mybir.ActivationFunctionType.Exp)
                            if w >= 0:
                                nc.gpsimd.affine_select(
                                    out=bd[:, off:off + 128],
                                    in_=bd[:, off:off + 128],
                                    compare_op=mybir.AluOpType.is_ge,
                                    fill=0.0, base=0, pattern=[[1, 128]],
                                    channel_multiplier=-1)
                            return bd, off

                        def emit_av(i, hx, bd_off):
                            bd, off = bd_off
                            nc.tensor.matmul(
                                avs[hx][:, off:512],
                                v_sb[:, i, 2 * hp + hx, :],
                                bd[:, off:512],
                                start=(i == 0), stop=(i == nst - 1))

                        pend = [emit_sc(0, 0), emit_sc(0, 64)]
                        for i in range(1, nst):
                            nxt0 = emit_sc(i, 0)
                            nxt1 = emit_sc(i, 64)
                            emit_av(i - 1, 0, pend[0])
                            emit_av(i - 1, 1, pend[1])
                            pend = [nxt0, nxt1]
                        emit_av(nst - 1, 0, pend[0])
                        emit_av(nst - 1, 1, pend[1])
                        # normalize: rows 0..63 are ctx^T, row 64 is denom
                        # (custom-DVE ops can't read PSUM at base_partition>0:
                        # stage the denom row to SBUF, reciprocal in place)
                        for hx in range(2):
                            po = hx * 64
                            rrow = stC.tile([1, 512], F32, tag="rr")
                            nc.vector.tensor_copy(rrow, avs[hx][64:65, :])
                            nc.vector.reciprocal_approx_fast(out=rrow, in_=rrow)
                            rb = stC.tile([64, 512], F32, tag="rb")
                            nc.gpsimd.partition_broadcast(rb[:, :], rrow[:, :])
                            nc.vector.tensor_mul(
                                ctxT[po:po + 64, hp, J * 512:(J + 1) * 512],
                                avs[hx][0:64, :], rb)
                    # Wo for this J-block's 4 token tiles, then RS chunk J
                    for ti in range(J * 4, J * 4 + 4):
                        wp = wops.tile([128, C], F32, tag="wp")
                        for dc in range(2):
                            for n0, n1 in ((0, 512), (512, 1024)):
                                nc.tensor.matmul(wp[:, n0:n1],
                                                 ctxT[:, dc, ti * 128:(ti + 1) * 128],
                                                 wo_sb[:, dc, n0:n1],
                                                 start=(dc == 0), stop=(dc == 1))
                        at = attnp.tile([128, C], BF, tag="at")
                        nc.vector.tensor_copy(at, wp)
                        nc.sync.dma_start(
                            out=rs_in[ti * 128:(ti + 1) * 128, :], in_=at)
                    t0, t1 = CHUNK_T[J]
                    o0, o1 = CHUNK_O[J]
                    if sim:
                        nc.sync.dma_start(
                            out=rs_out[o0:o1, :],
                            in_=rs_in[t0:t0 + (o1 - o0), :])
                    else:
                        nc.gpsimd.collective_compute(
                            "ReduceScatter", mybir.AluOpType.add,
                            replica_groups=RG,
                            ins=[rs_in[t0:t1, :]],
                            outs=[rs_out[o0:o1, :]])

            # ------- Phase E: LN2 prep + SwiGLU FFN in two token groups -----
            with tc.tile_pool(name="stE", bufs=2) as stE, \
                 tc.tile_pool(name="rsp", bufs=2) as rsp, \
                 tc.tile_pool(name="psE", bufs=2, space="PSUM") as psE, \
                 tc.tile_pool(name="gtmp", bufs=3) as gtmp, \
                 tc.tile_pool(name="psG", bufs=2, space="PSUM") as psG, \
                 tc.tile_pool(name="psW3", bufs=1, space="PSUM") as psW3:

                def emit_eprep(ck):
                    o0, o1 = CHUNK_O[ck]
                    tt = ck
                    rs_sb = rsp.tile([128, C], BF, tag="rs")
                    nc.gpsimd.dma_start(
                        out=rs_sb,
                        in_=rs_out[o0:o1, :].rearrange("(a p) c -> p a c", p=128))
                    o1s = out1[:, tt, :]
                    nc.vector.tensor_add(o1s, xr_sb[:, tt, :], rs_sb)
                    st = stE.tile([128, 2, 6], F32, tag="st")
                    nc.vector.bn_stats(out=st[:, 0, :], in_=o1s[:, 0:512])
                    nc.vector.bn_stats(out=st[:, 1, :], in_=o1s[:, 512:1024])
                    mv = stE.tile([128, 2], F32, tag="mv")
                    nc.vector.bn_aggr(out=mv, in_=st)
                    # sd_s = sqrt(var+eps)/S_X via activation scale/bias fold
                    sd = stE.tile([128, 1], F32, tag="sd")
                    nc.scalar.activation(out=sd, in_=mv[:, 1:2],
                                         func=mybir.ActivationFunctionType.Sqrt,
                                         bias=eps2c[:, 0:1], scale=1.0 / (S_X * S_X))
                    rstd = stE.tile([128, 1], F32, tag="rstd")
                    nc.vector.reciprocal(rstd, sd)  # = S_X * true rstd
                    rmu = stE.tile([128, 1], F32, tag="rmu")
                    nc.vector.tensor_mul(rmu, mv[:, 0:1], rstd)
                    hn2 = stE.tile([128, C], BF, tag="hn2")
                    nc.vector.tensor_scalar(hn2, o1s, rstd, rmu,
                                            mybir.AluOpType.mult,
                                            mybir.AluOpType.subtract)
                    for kc in range(NKC):
                        tp = psE.tile([128, 128], BF, tag="tpE")
                        nc.tensor.transpose(tp, hn2[:, kc * 128:(kc + 1) * 128],
                                            ident)
                        nc.vector.tensor_scalar_mul(
                            hn2T[:, kc, tt * 128:(tt + 1) * 128], tp, 1.0)

                def emit_ffn_group(g, tts, c0, c1):
                    ncols = c1 - c0
                    acc0f = psW3.tile([128, 2, 512], F32, tag="acc0", name=f"acc0_{g}")
                    acc1f = psW3.tile([128, 2, 512], F32, tag="acc1", name=f"acc1_{g}")
                    acc0 = acc0f[:, 0:len(tts), :]
                    acc1 = acc1f[:, 0:len(tts), :]
                    for fi in range(NF):
                        g12 = psG.tile([128, 2, ncols], F32, tag="g12")
                        g1 = g12[:, 0, :]
                        g2 = g12[:, 1, :]
                        for kcp in range(4):
                            nc.tensor.matmul(
                                g1, dr_pair(w12r[:, fi, kcp * 256:(kcp + 1) * 256]),
                                hn2T[:, 2 * kcp:2 * kcp + 2, c0:c1],
                                start=(kcp == 0), stop=(kcp == 3),
                                perf_mode=mybir.MatmulPerfMode.DoubleRow)
                        sil = gtmp.tile([128, ncols], BF, tag="sil")
                        nc.scalar.activation(out=sil, in_=g1,
                                             func=mybir.ActivationFunctionType.Silu,
                                             scale=1.0 / (s_w * S_X),
                                             bias=(bw1c[:, fi:fi + 1] if have_bw else 0.0))
                        for kcp in range(4):
                            nc.tensor.matmul(
                                g2, dr_pair(w12r[:, fi, 1024 + kcp * 256:
                                                 1024 + (kcp + 1) * 256]),
                                hn2T[:, 2 * kcp:2 * kcp + 2, c0:c1],
                                start=(kcp == 0), stop=(kcp == 3),
                                perf_mode=mybir.MatmulPerfMode.DoubleRow)
                        if have_bw:
                            nc.vector.tensor_scalar_add(g2, g2, bw2c[:, fi:fi + 1])
                        nc.vector.scalar_tensor_tensor(
                            out=g_sb[:, fi, c0:c1], in0=g2,
                            scalar=S_G / (s_w * S_X), in1=sil,
                            op0=mybir.AluOpType.mult, op1=mybir.AluOpType.mult)
                        if fi % 2 == 1:
                            for j, tt in enumerate(tts):
                                lhs = g_sb[:, fi - 1:fi + 1,
                                           tt * 128:(tt + 1) * 128]
                                nc.tensor.matmul(
                                    acc0[:, j, :], lhs, w3r[:, fi - 1:fi + 1, 0:512],
                                    start=(fi == 1), stop=(fi == NF - 1),
                                    perf_mode=mybir.MatmulPerfMode.DoubleRow)
                                nc.tensor.matmul(
                                    acc1[:, j, :], lhs, w3r[:, fi - 1:fi + 1, 512:1024],
                                    start=(fi == 1), stop=(fi == NF - 1),
                                    perf_mode=mybir.MatmulPerfMode.DoubleRow)
                    inv = 1.0 / (S_G * s_w3)
                    for j, tt in enumerate(tts):
                        nc.vector.scalar_tensor_tensor(
                            out=xr_sb[:, tt, 0:512], in0=acc0[:, j, :], scalar=inv,
                            in1=out1[:, tt, 0:512],
                            op0=mybir.AluOpType.mult, op1=mybir.AluOpType.add)
                        nc.vector.scalar_tensor_tensor(
                            out=xr_sb[:, tt, 512:1024], in0=acc1[:, j, :], scalar=inv,
                            in1=out1[:, tt, 512:1024],
                            op0=mybir.AluOpType.mult, op1=mybir.AluOpType.add)
                    outr = out[:].rearrange("(a p) c -> p a c", p=128)
                    nc.sync.dma_start(out=outr[:, tts[0]:tts[-1] + 1, :],
                                      in_=xr_sb[:, tts[0]:tts[-1] + 1, :])

                # group 0 fills the RS latency window; group 1 is gated by RS3
                emit_eprep(0)
                emit_eprep(1)
                emit_eprep(2)
                emit_ffn_group(0, (0, 1), 0, 256)
                emit_eprep(3)
                emit_ffn_group(1, (2, 3), 256, 512)
    nc.compile()
    return nc


def _prep(x, Wq, Wk, Wv, Wo, W1, W2, W3, gamma, beta):
    f32 = np.float32
    scale = f32(1.0 / np.sqrt(HS))
    gcol = gamma.astype(f32)[:, None]
    w1p = np.zeros((C, HIDP), f32)
    w1p[:, :HID] = W1
    w2p = np.zeros((C, HIDP), f32)
    w2p[:, :HID] = W2
    w3p = np.zeros((HIDP, C), f32)
    w3p[:HID, :] = W3
    w1g = (gcol * w1p).reshape(NKC, 128, NF, 128).transpose(2, 1, 0, 3).reshape(NF, 128, C)
    w2g = (gcol * w2p).reshape(NKC, 128, NF, 128).transpose(2, 1, 0, 3).reshape(NF, 128, C)
    w12 = np.concatenate([w1g, w2g], axis=2)  # [NF, 128, 2048]
    s_w = float(2.0 ** math.floor(math.log2(240.0 / max(np.abs(w12).max(), 1e-30))))
    wf12 = (w12 * s_w).astype(E4)
    w3r = w3p.reshape(NF, 128, C)
    s_w3 = float(2.0 ** math.floor(math.log2(240.0 / max(np.abs(w3r).max(), 1e-30))))
    wf3 = (w3r * s_w3).astype(E4)
    bw1 = (beta.astype(f32) @ w1p).astype(f32)
    bw2 = ((beta.astype(f32) @ w2p) * (s_w * S_X)).astype(f32)
    in_maps = []
    for c in range(8):
        b, r = c // 4, c % 4
        hh = slice(r * HPC, (r + 1) * HPC)
        # per-head [C, HS] blocks -> [C, 256] column groups
        qc = Wq[hh].transpose(1, 0, 2).reshape(C, 256).astype(f32) * scale
        kc = Wk[hh].transpose(1, 0, 2).reshape(C, 256).astype(f32)
        vc = Wv[hh].transpose(1, 0, 2).reshape(C, 256).astype(f32)
        wcat = np.concatenate([qc, kc, vc], axis=1)  # [C, 768], scale folded in q
        wq_g = gcol * wcat
        gws = wq_g.sum(axis=0).astype(f32)
        bwq = (beta.astype(f32) @ wcat).astype(f32)
        xb = x[b].astype(f32)
        xres_rows = np.concatenate(
            [xb[512 * J + 128 * r: 512 * J + 128 * (r + 1)] for J in range(4)])
        m = {
            "xT": np.ascontiguousarray(xb.T).astype(BF16),
            "xres": np.ascontiguousarray(xres_rows),
            "wqkv": wq_g.astype(BF16),
            "gws": gws,
            "wo": np.ascontiguousarray(Wo[r * 256:(r + 1) * 256, :]).astype(BF16),
            "wf12": wf12,
            "wf3": wf3,
        }
        have_bw = bool(np.any(beta != 0))
        if have_bw:
            m["bw"] = bwq
            m["bw1"] = bw1
            m["bw2"] = bw2
        in_maps.append(m)
    return in_maps, have_bw, s_w, s_w3


def kernel(x, Wq, Wk, Wv, Wo, W1, W2, W3, gamma, beta, _bench=None):
    x = np.asarray(x)
    in_maps, have_bw, s_w, s_w3 = _prep(
        np.asarray(x), np.asarray(Wq), np.asarray(Wk),
        np.asarray(Wv), np.asarray(Wo), np.asarray(W1),
        np.asarray(W2), np.asarray(W3),
        np.asarray(gamma), np.asarray(beta))
    key = ("k", have_bw, s_w, s_w3)
    if key not in _cache:
        _cache[key] = _build(have_bw, s_w, s_w3)
    nc = _cache[key]
    kw = dict(_bench) if _bench else {}
    res = run_bass_kernel_spmd(nc, in_maps, list(range(8)), **kw)
    outf = np.empty((B, T, C), np.float32)
    for c in range(8):
        b, r = c // 4, c % 4
        o = res.results[c]["out"]
        for J in range(4):
            outf[b, 512 * J + 128 * r: 512 * J + 128 * (r + 1)] = \
                o[J * 128:(J + 1) * 128]
    if _bench is not None:
        kernel.last_results = res
    return outf
